# revision 8
# baseline (speedup 1.0000x reference)
"""Tensor-parallel GPT-J-style attention block on 8 TRN2 NeuronCores, v2.

Sharding: TP over heads (2 heads/core) for QKV + attention, AllToAll to
token-shard before the output projection (so proj has no cross-core
reduction). v1's on-device data-plumbing is replaced by host-side input
staging:
  - hidden_states is transposed + tiled on host and replicated to all
    cores (hid_bt), so there is no on-device transpose and no hidT
    AllGather.
  - w_out is replicated to all cores in a [nt][kt][128,512] blocked
    layout (no w_out AllGather); w_qkv is column-sharded per core in a
    [pass][kt][128,768] blocked layout.
  - rope cos/sin tables are computed on host ([cos;cos] and [sin;-sin]
    stacked to 128 partitions so rope is 5 vector ops per tile).
  - v is produced directly in [tok, d] orientation by a second matmul
    group (hidT tile stationary, w_v moving), eliminating PE transposes.
The only on-device collectives are the two per-head AllToAlls (4 MB
each), issued right after each head's attention so they hide under the
next phase. All matmuls are float32r (full PE rate at moving dim >=256).
"""
import math
import sys

import numpy as np

try:
    import concourse.bass  # noqa: F401
except ImportError:
    sys.path.insert(0, "/opt/trn_rl_repo")

import concourse.mybir as mybir
import concourse.tile as tile
from concourse import bacc
from concourse.bass_utils import run_bass_kernel_spmd
from concourse.masks import make_upper_triangular

dt = mybir.dt

N_CORES = 8
B = 4
NH = 16
D = 256
HID = NH * D  # 4096
ROT = D // 2  # 128
RH = ROT // 2  # 64
HPC = NH // N_CORES  # 2 heads per core
TOK = B * 1024
TS = TOK // N_CORES  # 512 tokens/core for proj + output
TBW = 256  # QKV token-block width
NTB = TOK // TBW  # 16
KT = HID // 128  # 32 contraction tiles
PCOLS = 3 * D  # 768 qkv columns per pass (one head)
SCALE = 1.0 / math.sqrt(D)
ROPE_BASE = 10000.0

_BUILD_CACHE = {}


def build(S, reps=1, phases=('qkv', 'attn', 'proj')):
    assert B * S == TOK
    f32, f32r = dt.float32, dt.float32r
    RG = [list(range(N_CORES))]
    NKT8 = S // 128  # 8 k-token tiles per attention instance
    QW = 512
    NQH = S // QW  # 2

    nc = bacc.Bacc("TRN2", target_bir_lowering=False, debug=False,
                   num_devices=N_CORES)

    # ---- I/O (host-blocked layouts; see make_in_maps)
    hid_in = nc.dram_tensor("hid_bt", [NTB * KT * 128, TBW], f32r, kind="ExternalInput")
    wqkv_in = nc.dram_tensor("wqkv_b", [HPC * KT * 128, PCOLS], f32r, kind="ExternalInput")
    wout_in = nc.dram_tensor("wout_b", [(HID // 512) * KT * 128, 512], f32r, kind="ExternalInput")
    cc_in = nc.dram_tensor("cc_t", [ROT, TOK], f32, kind="ExternalInput")
    ss_in = nc.dram_tensor("ss_t", [ROT, TOK], f32, kind="ExternalInput")
    out_f = nc.dram_tensor("out_f", [TS, HID], f32, kind="ExternalOutput")

    # ---- internal DRAM
    # qk_d rows: pass p at 512p + 128*ct, ct in {q_rot, q_pass, k_rot, k_pass}
    qk_d = nc.dram_tensor("qk_d", [HPC * 512, TOK], f32r)
    v_d = [nc.dram_tensor(f"v_d{p}", [TOK, D], f32r) for p in range(HPC)]
    a2a_in = [nc.dram_tensor(f"a2a_in{h}", [N_CORES, D, TS], f32r) for h in range(HPC)]
    a2a_out = [nc.dram_tensor(f"a2a_out{h}", [N_CORES, D, TS], f32r)
               for h in range(HPC)]

    with tile.TileContext(nc) as tc:
        with tc.tile_pool(name="const", bufs=1) as cpool:
            ones_f = cpool.tile([128, 1], f32)
            nc.vector.memset(ones_f[:], 1.0)
            ones_r = cpool.tile([128, 1], f32r)
            nc.vector.tensor_copy(out=ones_r[:], in_=ones_f[:])
            tri_f = cpool.tile([128, 128], f32)
            make_upper_triangular(nc, tri_f[:], val=1.0, diag=True)
            tri_r = cpool.tile([128, 128], f32r)
            nc.vector.tensor_copy(out=tri_r[:], in_=tri_f[:])

            for rep in range(reps):
                for p in range(HPC if 'qkv' in phases or 'attn' in phases else 0):
                    # ---- QKV pass p: q/k in [col, tok] (transposed), v in [tok, col]
                    with tc.tile_pool(name=f"qkv_w{p}_{rep}", bufs=1) as wpool, \
                         tc.tile_pool(name=f"qkv_a{p}_{rep}", bufs=2) as apool, \
                         tc.tile_pool(name=f"qkv_st{p}_{rep}", bufs=2) as spool, \
                         tc.tile_pool(name=f"qkv_ps{p}_{rep}", bufs=2, space="PSUM") as pspool:
                        wres = wpool.tile([128, KT * PCOLS], f32r)
                        for kt in range(KT):
                            r0 = (p * KT + kt) * 128
                            nc.sync.dma_start(
                                out=wres[:, PCOLS * kt:PCOLS * (kt + 1)],
                                in_=wqkv_in.ap()[r0:r0 + 128, :])
                        for tb in range(NTB if 'qkv' in phases else 0):
                            t0 = TBW * tb
                            acts = apool.tile([128, KT * TBW], f32r, tag="acts",
                                              name=f"acts_{p}_{tb}_r{rep}")
                            for kt in range(KT):
                                r0 = (tb * KT + kt) * 128
                                nc.sync.dma_start(
                                    out=acts[:, TBW * kt:TBW * (kt + 1)],
                                    in_=hid_in.ap()[r0:r0 + 128, :])
                            # 2 q/k psums ([col,tok], 2 col-tiles each) + 1 v psum
                            qk_ps = [pspool.tile([128, 2 * TBW], f32, tag=f"qkps{i}",
                                                 name=f"qkps{i}_{p}_{tb}_r{rep}")
                                     for i in range(2)]
                            v_ps = pspool.tile([128, 2 * TBW], f32, tag="vps",
                                               name=f"vps_{p}_{tb}_r{rep}")
                            # NOTE: two accumulation streams share each PSUM
                            # bank (column halves). start=True clears the
                            # has_written bits of the WHOLE bank, so only the
                            # first matmul touching a bank may carry it; the
                            # second stream's kt==0 matmul overwrites via the
                            # cleared bits (start=False) and accumulates after.
                            for kt in range(KT):
                                a = acts[:, TBW * kt:TBW * (kt + 1)]
                                w = wres[:, PCOLS * kt:PCOLS * (kt + 1)]
                                sp = kt == KT - 1
                                for i in range(2):
                                    for j in range(2):
                                        nc.tensor.matmul(
                                            out=qk_ps[i][:, TBW * j:TBW * (j + 1)],
                                            lhsT=w[:, 128 * (2 * i + j):128 * (2 * i + j + 1)],
                                            rhs=a, start=(kt == 0 and j == 0), stop=sp)
                                for s in range(2):
                                    nc.tensor.matmul(
                                        out=v_ps[:, TBW * s:TBW * (s + 1)],
                                        lhsT=a[:, 128 * s:128 * (s + 1)],
                                        rhs=w[:, 512:768],
                                        start=(kt == 0 and s == 0), stop=sp)
                            # drain q/k: rope on col-tile 0 of q and k, copy on tile 1
                            cs_ = spool.tile([ROT, TBW], f32, tag="cs",
                                             name=f"cs_{p}_{tb}_r{rep}")
                            nc.sync.dma_start(out=cs_[:], in_=cc_in.ap()[:, t0:t0 + TBW])
                            sn_ = spool.tile([ROT, TBW], f32, tag="sn",
                                             name=f"sn_{p}_{tb}_r{rep}")
                            nc.sync.dma_start(out=sn_[:], in_=ss_in.ap()[:, t0:t0 + TBW])
                            for i in range(2):  # 0 = q, 1 = k
                                ps = qk_ps[i]
                                rot = ps[:, 0:TBW]
                                dst = spool.tile([128, TBW], f32r, tag=f"dst{i}",
                                                 name=f"dst{i}_{p}_{tb}_r{rep}")
                                swp = spool.tile([128, TBW], f32, tag=f"swp{i}",
                                                 name=f"swp{i}_{p}_{tb}_r{rep}")
                                nc.vector.tensor_copy(out=swp[0:RH, :], in_=rot[RH:ROT, :])
                                nc.vector.tensor_copy(out=swp[RH:ROT, :], in_=rot[0:RH, :])
                                t1 = spool.tile([128, TBW], f32, tag=f"t1_{i}",
                                                name=f"t1_{i}_{p}_{tb}_r{rep}")
                                nc.vector.tensor_mul(t1[:], rot, cs_[:])
                                t2 = spool.tile([128, TBW], f32, tag=f"t2_{i}",
                                                name=f"t2_{i}_{p}_{tb}_r{rep}")
                                nc.vector.tensor_mul(t2[:], swp[:], sn_[:])
                                nc.vector.tensor_sub(dst[:], t1[:], t2[:])
                                nc.sync.dma_start(
                                    out=qk_d.ap()[512 * p + 256 * i:512 * p + 256 * i + 128,
                                                  t0:t0 + TBW],
                                    in_=dst[:])
                                pas = spool.tile([128, TBW], f32r, tag=f"pas{i}",
                                                 name=f"pas{i}_{p}_{tb}_r{rep}")
                                nc.scalar.copy(out=pas[:], in_=ps[:, TBW:2 * TBW])
                                nc.sync.dma_start(
                                    out=qk_d.ap()[512 * p + 256 * i + 128:512 * p + 256 * i + 256,
                                                  t0:t0 + TBW],
                                    in_=pas[:])
                            # drain v ([tok, col], 2 token sub-tiles)
                            vst = spool.tile([128, 2 * TBW], f32r, tag="vst",
                                             name=f"vst_{p}_{tb}_r{rep}")
                            nc.scalar.copy(out=vst[:], in_=v_ps[:])
                            for s in range(2):
                                nc.sync.dma_start(
                                    out=v_d[p].ap()[t0 + 128 * s:t0 + 128 * (s + 1), :],
                                    in_=vst[:, TBW * s:TBW * (s + 1)])

                    # ---- attention for head p, all batches
                    with tc.tile_pool(name=f"att_in_{p}_{rep}", bufs=2) as ain_pool, \
                         tc.tile_pool(name=f"att_pr_{p}_{rep}", bufs=1) as apr_pool, \
                         tc.tile_pool(name=f"att_o_{p}_{rep}", bufs=2) as aout_pool, \
                         tc.tile_pool(name=f"att_sc_{p}_{rep}", bufs=2, space="PSUM") as scps_pool, \
                         tc.tile_pool(name=f"att_av_{p}_{rep}", bufs=1, space="PSUM") as avps_pool:
                        for b in range(B if 'attn' in phases else 0):
                            tok0 = S * b

                            def load_qk(off, nm):
                                ts_ = []
                                for dtile in range(2):
                                    t = ain_pool.tile([128, S], f32r, tag=f"{nm}{dtile}")
                                    nc.sync.dma_start(
                                        out=t[:],
                                        in_=qk_d.ap()[off + 128 * dtile:off + 128 * (dtile + 1),
                                                      tok0:tok0 + S])
                                    ts_.append(t)
                                return ts_

                            qT = load_qk(512 * p, "q")
                            kT = load_qk(512 * p + 256, "k")
                            vt = ain_pool.tile([128, NKT8 * D], f32r, tag="vt")
                            for kt8 in range(NKT8):
                                nc.sync.dma_start(
                                    out=vt[:, D * kt8:D * (kt8 + 1)],
                                    in_=v_d[p].ap()[tok0 + 128 * kt8:tok0 + 128 * (kt8 + 1), :])

                            # scoresT -> exp -> probsT (causal: q >= 128*kt8 only)
                            probsT = []
                            for kt8 in range(NKT8):
                                pr = apr_pool.tile([128, S], f32r, tag=f"pr{kt8}")
                                ql = 128 * kt8
                                q0 = ql
                                while q0 < S:
                                    wch = min(512, S - q0)
                                    pss = scps_pool.tile([128, QW], f32, tag="scps")
                                    for dtile in range(2):
                                        nc.tensor.matmul(
                                            out=pss[:, 0:wch],
                                            lhsT=kT[dtile][:, ql:ql + 128],
                                            rhs=qT[dtile][:, q0:q0 + wch],
                                            start=(dtile == 0), stop=(dtile == 1))
                                    nc.scalar.activation(
                                        out=pr[:, q0:q0 + wch], in_=pss[:, 0:wch],
                                        func=mybir.ActivationFunctionType.Exp, scale=SCALE)
                                    q0 += wch
                                nc.vector.tensor_mul(pr[:, ql:ql + 128],
                                                     pr[:, ql:ql + 128], tri_r[:])
                                probsT.append(pr)

                            # PV + denominator (causal blocks only)
                            ps_av = [[avps_pool.tile([128, QW], f32, tag=f"av{d}{q}",
                                                     name=f"av{d}{q}_{p}_{b}_r{rep}")
                                      for q in range(NQH)] for d in range(2)]
                            ps_sum = [avps_pool.tile([1, QW], f32, tag=f"sm{q}",
                                                     name=f"sm{q}_{p}_{b}_r{rep}")
                                      for q in range(NQH)]
                            pv_work = {}
                            for qh in range(NQH):
                                q0, q1 = QW * qh, QW * (qh + 1)
                                last_kt = min(NKT8 - 1, (q1 - 1) // 128)
                                pv_work[qh] = [
                                    (kt8, q0, q1, max(128 * kt8, q0),
                                     kt8 == 0, kt8 == last_kt)
                                    for kt8 in range(NKT8)
                                    if max(128 * kt8, q0) < q1]
                            for dtile in range(2):
                                for qh in range(NQH):
                                    for kt8, q0, q1, lo, st, sp in pv_work[qh]:
                                        nc.tensor.matmul(
                                            out=ps_av[dtile][qh][:, lo - q0:q1 - q0],
                                            lhsT=vt[:, D * kt8 + 128 * dtile:D * kt8 + 128 * (dtile + 1)],
                                            rhs=probsT[kt8][:, lo:q1], start=st, stop=sp)
                            for qh in range(NQH):
                                for kt8, q0, q1, lo, st, sp in pv_work[qh]:
                                    nc.tensor.matmul(out=ps_sum[qh][:, lo - q0:q1 - q0],
                                                     lhsT=ones_r[:],
                                                     rhs=probsT[kt8][:, lo:q1],
                                                     start=st, stop=sp)

                            # normalize: 1/denominator broadcast via GpSimd
                            sums_sb = aout_pool.tile([1, S], f32, tag="sums")
                            for qh in range(NQH):
                                nc.scalar.copy(out=sums_sb[:, QW * qh:QW * (qh + 1)],
                                               in_=ps_sum[qh][:])
                            recip = aout_pool.tile([1, S], f32, tag="recip")
                            nc.vector.reciprocal(out=recip[:], in_=sums_sb[:])
                            rbc = aout_pool.tile([128, S], f32, tag="rbc")
                            nc.gpsimd.partition_broadcast(rbc[:], recip[0:1, :])
                            for dtile in range(2):
                                att_sb = aout_pool.tile([128, S], f32r, tag=f"attn{dtile}")
                                for qh in range(NQH):
                                    q0, q1 = QW * qh, QW * (qh + 1)
                                    nc.vector.tensor_mul(att_sb[:, q0:q1],
                                                         ps_av[dtile][qh][:],
                                                         rbc[:, q0:q1])
                                for u in range(S // TS):
                                    dest = (S * b) // TS + u
                                    nc.sync.dma_start(
                                        out=a2a_in[p].ap()[dest,
                                                           128 * dtile:128 * (dtile + 1), :],
                                        in_=att_sb[:, TS * u:TS * (u + 1)])
                        if 'attn' in phases:
                            nc.gpsimd.collective_compute(
                                "AllToAll", mybir.AluOpType.bypass, replica_groups=RG,
                                ins=[a2a_in[p].ap().opt()], outs=[a2a_out[p].ap().opt()])

                # ---- output projection for own token slice
                NNT = HID // 512 if 'proj' in phases else 0
                kts = [(hl, src, sub) for hl in range(HPC)
                       for src in range(N_CORES) for sub in range(2)]
                PKB = 4
                with tc.tile_pool(name=f"op_a_{rep}", bufs=1) as oa_pool, \
                     tc.tile_pool(name=f"op_w_{rep}", bufs=8) as ow_pool, \
                     tc.tile_pool(name=f"op_f_{rep}", bufs=3) as of_pool, \
                     tc.tile_pool(name=f"op_ps_{rep}", bufs=1, space="PSUM") as ops_pool:
                    am = []
                    for hl, src, sub in (kts if 'proj' in phases else []):
                        t = oa_pool.tile([128, TS], f32r, tag=f"am{hl}_{src}_{sub}")
                        nc.sync.dma_start(
                            out=t[:],
                            in_=a2a_out[hl].ap()[src, 128 * sub:128 * (sub + 1), :])
                        am.append(t)
                    NMT = TS // 128  # 4
                    for nt in range(NNT):
                        ps_f = [ops_pool.tile([128, 512], f32, tag=f"f{mt}",
                                              name=f"f{mt}_{nt}_r{rep}")
                                for mt in range(NMT)]
                        for kb in range(KT // PKB):
                            wblk = ow_pool.tile([128, PKB * 512], f32r, tag="wblk",
                                                name=f"wblk_{nt}_{kb}_r{rep}")
                            for i in range(PKB):
                                r0 = (nt * KT + kb * PKB + i) * 128
                                nc.sync.dma_start(
                                    out=wblk[:, 512 * i:512 * (i + 1)],
                                    in_=wout_in.ap()[r0:r0 + 128, :])
                            for mt in range(NMT):
                                for i in range(PKB):
                                    ki = kb * PKB + i
                                    nc.tensor.matmul(
                                        out=ps_f[mt][:],
                                        lhsT=am[ki][:, 128 * mt:128 * (mt + 1)],
                                        rhs=wblk[:, 512 * i:512 * (i + 1)],
                                        start=(ki == 0), stop=(ki == KT - 1))
                        for mt in range(NMT):
                            fo = of_pool.tile([128, 512], f32, tag="fo")
                            nc.scalar.copy(out=fo[:], in_=ps_f[mt][:])
                            nc.sync.dma_start(
                                out=out_f.ap()[128 * mt:128 * (mt + 1),
                                               512 * nt:512 * (nt + 1)],
                                in_=fo[:])

    nc.compile()
    return nc


def get_nc(S):
    if S not in _BUILD_CACHE:
        _BUILD_CACHE[S] = build(S)
    return _BUILD_CACHE[S]


def make_in_maps(position_ids, hidden_states, w_qkv, w_out):
    S = hidden_states.shape[1]
    flat = np.asarray(hidden_states, dtype=np.float32).reshape(TOK, HID)
    w_qkv = np.asarray(w_qkv, dtype=np.float32)
    w_out = np.asarray(w_out, dtype=np.float32)

    # hidT tiled: [tb][kt][128, TBW] (replicated to every core)
    hid_bt = np.ascontiguousarray(
        flat.T.reshape(KT, 128, NTB, TBW).transpose(2, 0, 1, 3)
    ).reshape(NTB * KT * 128, TBW)

    # w_out blocked: kt order (hl, src, sub) to match a2a layout; [nt][kt][128,512]
    wo = w_out.reshape(N_CORES, HPC, 2, 128, HID).transpose(1, 0, 2, 3, 4)
    wout_b = np.ascontiguousarray(
        wo.reshape(KT, 128, HID // 512, 512).transpose(2, 0, 1, 3)
    ).reshape((HID // 512) * KT * 128, 512)

    # rope tables from position_ids (identical rows across batch)
    pos = np.asarray(position_ids).astype(np.float32).reshape(TOK)
    invf = 1.0 / (ROPE_BASE ** (np.arange(0, ROT, 2, dtype=np.float32) / ROT))
    ang = invf[:, None] * pos[None, :]  # [RH, TOK]
    cos, sin = np.cos(ang, dtype=np.float32), np.sin(ang, dtype=np.float32)
    cc_t = np.ascontiguousarray(np.concatenate([cos, cos], axis=0))
    ss_t = np.ascontiguousarray(np.concatenate([sin, -sin], axis=0))

    in_maps = []
    for c in range(N_CORES):
        # per-pass head g = 2c + p: cols [q_g | k_g | v_g], blocked [p][kt][128, 768]
        wp = []
        for p_ in range(HPC):
            g = HPC * c + p_
            cols = np.concatenate([w_qkv[:, D * g:D * (g + 1)],
                                   w_qkv[:, HID + D * g:HID + D * (g + 1)],
                                   w_qkv[:, 2 * HID + D * g:2 * HID + D * (g + 1)]],
                                  axis=1)  # [HID, 768]
            wp.append(cols.reshape(KT, 128, PCOLS))
        wqkv_b = np.ascontiguousarray(np.stack(wp, axis=0)).reshape(HPC * KT * 128, PCOLS)
        in_maps.append({
            "hid_bt": hid_bt,
            "wqkv_b": wqkv_b,
            "wout_b": wout_b,
            "cc_t": cc_t,
            "ss_t": ss_t,
        })
    return in_maps


def kernel(position_ids, hidden_states, w_qkv, w_out):
    S = hidden_states.shape[1]
    nc = get_nc(S)
    in_maps = make_in_maps(position_ids, hidden_states, w_qkv, w_out)
    res = run_bass_kernel_spmd(nc, in_maps, list(range(N_CORES)))
    out = np.concatenate([res.results[c]["out_f"] for c in range(N_CORES)], axis=0)
    return out.reshape(B, S, HID).astype(np.float32)
